# revision 58
# baseline (speedup 1.0000x reference)
"""Multi-head attention forward (B=2, N=2048, C=768, H=12) on 8 TRN2 cores.

Sharding: core = b*4 + g handles batch b, heads 3g..3g+2. Each core computes
qkv for its heads (all matmul operands bf16), full N x N logits per head in
[key, query] orientation (key mask folds into the exp bias), exp on the
Activation engine into bf16 SBUF tiles, then a "flipped" PV: the exp tile is
the stationary operand and the moving operand is the 65-wide [v | ones]
block, so each 128-query chain costs 65 cycles/k-tile and the softmax
denominator rides in column 64. Normalization is a per-partition
reciprocal+scale on DVE; a single xbar DMA transpose per (head-pair,
query-half) flips the [query, dim] chains into the d-major layout the output
projection needs. Host sums the 4 per-group partial projections per batch
and adds the bias.

Work is laid out as 6 sequential units (3 heads x 2 query halves). Unit U's
logits/exp loop is interleaved (in PE program order) with unit U-1's PV
chains plus qkv/v/proj filler passes so the PE never waits on the
Activation engine, which is the per-unit long pole.
"""

import numpy as np
import ml_dtypes

from concourse import bacc
import concourse.mybir as mybir
import concourse.tile as tile
from concourse.bass_utils import run_bass_kernel_spmd

B, N, C = 2, 2048, 768
H, DH = 12, 64
G = 4           # head groups (cores per batch)
HPC = 3         # heads per core
P = 128
KT = C // P     # 6 contraction tiles over channels
NMT = N // P    # 16 key tiles
W = 1024        # query-half width
NQT = N // P    # 16 query tiles (128 each)
JT = W // P     # 8 query tiles per half
VB = HPC * (DH + 1)   # 195: per-mt v block [v0|1|v1|1|v2|1]
SCALE = float(DH) ** -0.5

TRACE = False
LAST_EXEC_NS = None
LAST_RESULTS = None

_nc_cache = {}

f32 = mybir.dt.float32
bf16 = mybir.dt.bfloat16
i32 = mybir.dt.int32

# Schraudolph-style exp: bits = round(t*log2e*2^23 + 127*2^23) gives
# y = 2^floor(t') * (1+w), w = frac(t'); the fused DVE op recovers w from the
# mantissa bits and applies g(w) ~= 1 + EXP_C*(w - w^2) (max rel err ~5e-3).
EXP_SCALE = float(2.0**23 / np.log(2.0))
EXP_BIAS = float(127 * 2**23)
EXP_C = -0.2354794859640161
MANT_MASK = float(np.frombuffer(np.uint32(0x007FFFFF).tobytes(), np.float32)[0])

_EXP_FIX = None


def _get_exp_fix():
    """Register (once) a fused custom DVE op: given y = bitcast(f32) of the
    Schraudolph int32, compute w = frac from the mantissa bits and return
    y * (1 + C1*(w - w^2)).  C0 = mantissa mask, C1 = correction coeff."""
    global _EXP_FIX
    if _EXP_FIX is not None:
        return _EXP_FIX
    import concourse.dve_ops as dops
    from concourse.dve_spec import (
        Spec, Src0, C0, C1, One, sq, lower, Bin, AluOp,
    )
    from concourse.dve_uop import DveOpSpec

    def ref(in0, in1, c0, c1, c2):
        bits = in0.view(np.int32) if hasattr(in0, "view") else in0
        m = (bits & np.float32(c0).view(np.int32)) | np.float32(1.0).view(np.int32)
        w = m.view(np.float32) - 1.0
        return in0 * (1.0 + c1 * (w - w * w))

    v = Bin(AluOp.BITWISE_OR, Bin(AluOp.BITWISE_AND, Src0, C0), One)
    w = v - One
    spec = Spec(body=Src0 * (One + C1 * (w - sq(w))), reference=ref)
    shas = {}
    for ver in ("v3", "v4"):
        try:
            s = DveOpSpec(name="EXP_FIX_ANT", opcode=None, uops=lower(spec, ver=ver),
                          rd1_en=False)
            shas[ver] = s.sha(ver)
        except Exception:
            pass
    op = dops.DveOp("EXP_FIX_ANT", spec, subdim=False, uops_sha=shas)
    dops.OPS.append(op)
    dops.CUSTOM_DVE_SPECS[op.name] = op.spec
    dops._SUB_OPCODE_FOR_NAME[op.name] = dops._CUSTOM_DVE_ROW_BASE + len(dops.OPS) - 1
    _EXP_FIX = op
    return op


def _build(reps=1):
    nc = bacc.Bacc("TRN2", debug=False)

    xT = nc.dram_tensor("xT", [C, N], bf16, kind="ExternalInput")
    wqkT = nc.dram_tensor("wqkT", [C, 3 * P], bf16, kind="ExternalInput")
    wvT = nc.dram_tensor("wvT", [C, HPC * DH], bf16, kind="ExternalInput")
    wpTA = nc.dram_tensor("wpTA", [P, C], bf16, kind="ExternalInput")
    wpTB = nc.dram_tensor("wpTB", [DH, C], bf16, kind="ExternalInput")
    mbias = nc.dram_tensor("mbias", [P, NMT], f32, kind="ExternalInput")
    mbias2 = nc.dram_tensor("mbias2", [P, NMT], f32, kind="ExternalInput")
    y = nc.dram_tensor("y", [N, C], bf16, kind="ExternalOutput")
    _get_exp_fix()

    with tile.TileContext(nc) as tc:
        with (
            tc.tile_pool(name="big", bufs=1) as big,
            tc.tile_pool(name="exps", bufs=34) as exps,
            tc.tile_pool(name="norms", bufs=2) as norms,
            tc.tile_pool(name="rcs", bufs=6) as rcs,
            tc.tile_pool(name="ys", bufs=8) as ys,
            tc.tile_pool(name="pa", bufs=3, space="PSUM") as pa,
            tc.tile_pool(name="ch", bufs=2, space="PSUM") as ch,
        ):
            for _ in range(reps):
                body(nc, tc, big, exps, norms, rcs, ys, pa, ch,
                     xT, wqkT, wvT, wpTA, wpTB, mbias, mbias2, y)

    nc.compile()
    return nc


def body(nc, tc, big, exps, norms, rcs, ys, pa, ch,
         xT, wqkT, wvT, wpTA, wpTB, mbias, mbias2, y):
    xT_sb = big.tile([P, KT * N], bf16, tag="xT", name="xT_sb")
    wqk_sb = big.tile([P, KT * 3 * P], bf16, tag="wqk", name="wqk_sb")
    wv_sb = big.tile([P, KT * HPC * DH], bf16, tag="wv", name="wv_sb")
    wpA = big.tile([P, C], bf16, tag="wpA", name="wpA")
    wpB = big.tile([DH, C], bf16, tag="wpB", name="wpB")
    mb_sb = big.tile([P, NMT], f32, tag="mb", name="mb_sb")
    mb2_sb = big.tile([P, NMT], f32, tag="mb2", name="mb2_sb")
    qA = big.tile([P, N], bf16, tag="qA", name="qA")    # q d-major h0|h1
    kA = big.tile([P, N], bf16, tag="kA", name="kA")    # k d-major h0|h1
    tB = big.tile([P, N], bf16, tag="tB", name="tB")    # q2 (0:64) | k2 (64:128)
    kB = big.tile([DH, N], bf16, tag="kB", name="kB")   # k2 moved to base 0
    v_sb = big.tile([P, NMT * VB], bf16, tag="v", name="v_sb")
    atA = big.tile([P, N], bf16, tag="atA", name="atA")  # d-major attn h0|h1
    atB = big.tile([DH, N], bf16, tag="atB", name="atB")  # d-major attn h2

    # --- input DMAs, ordered by first use (wqk + x chunks 0/1 gate unit 0) ---
    xTv = xT[:, :].rearrange("(k p) n -> p k n", p=P)
    xsv = xT_sb[:].rearrange("p (k n) -> p k n", n=N)

    def dma_x(n0, n1):
        nc.sync.dma_start(xsv[:, :, n0:n1], xTv[:, :, n0:n1])

    wqkv = wqk_sb[:].rearrange("p (k c) -> p k c", c=3 * P)
    wqkTv = wqkT[:, :].rearrange("(k p) c -> p k c", p=P)
    nc.sync.dma_start(wqkv[:, :, 0 : 2 * P], wqkTv[:, :, 0 : 2 * P])
    dma_x(0, 256)
    dma_x(256, 512)
    nc.sync.dma_start(mb_sb[:], mbias[:, :])
    nc.sync.dma_start(mb2_sb[:], mbias2[:, :])
    dma_x(512, 1024)
    nc.sync.dma_start(wqkv[:, :, 2 * P : 3 * P], wqkTv[:, :, 2 * P : 3 * P])
    dma_x(1024, 1536)
    dma_x(1536, 2048)
    nc.sync.dma_start(
        wv_sb[:].rearrange("p (k c) -> p k c", c=HPC * DH),
        wvT[:, :].rearrange("(k p) c -> p k c", p=P),
    )
    nc.sync.dma_start(wpA[:], wpTA[:, :])
    nc.sync.dma_start(wpB[:], wpTB[:, :])

    # ones columns of the v blocks (static); warmup source tile
    ones_w = big.tile([P, 512], bf16, tag="onesw", name="ones_w")
    nc.gpsimd.memset(ones_w[:], 1.0)
    nc.gpsimd.memset(
        v_sb[:].rearrange("p (a c) -> p a c", c=DH + 1)[:, :, DH : DH + 1], 1.0
    )

    # keep the PE busy while input DMAs land so the p-state ramp finishes
    # before real work starts (results are never read)
    def warmup(n):
        ps = pa.tile([P, 512], f32, tag="pa", name="ps_warm")
        for _ in range(n):
            nc.tensor.matmul(
                ps[:, :], ones_w[:, 0:P], ones_w[:, :], start=True, stop=True
            )

    # --- qk pass: d-major q/k for one n-range chunk ---
    # wqk col blocks: [wq_h0|wq_h1][wk_h0|wk_h1][wq_h2|wk_h2]
    def qk_pass(col0, rows, n0, n1, dest):
        ps = pa.tile([P, 512], f32, tag="pa", name="ps_qk")
        nw = n1 - n0
        for kt in range(KT):
            nc.tensor.matmul(
                ps[0:rows, 0:nw],
                wqk_sb[:, kt * 3 * P + col0 : kt * 3 * P + col0 + rows],
                xT_sb[:, kt * N + n0 : kt * N + n1],
                start=(kt == 0),
                stop=(kt == KT - 1),
            )
        nc.vector.tensor_copy(dest[:, n0:n1], ps[0:rows, 0:nw])

    # --- v pass: keys-major v for one 128-key tile ---
    def v_tile(mt):
        ps = pa.tile([P, HPC * DH], f32, tag="pa", name="ps_v")
        for kt in range(KT):
            nc.tensor.matmul(
                ps[:, :],
                xT_sb[:, kt * N + mt * P : kt * N + (mt + 1) * P],
                wv_sb[:, kt * HPC * DH : (kt + 1) * HPC * DH],
                start=(kt == 0),
                stop=(kt == KT - 1),
            )
        nc.vector.tensor_copy(
            v_sb[:].rearrange("p (m a c) -> p (m a) c", c=DH + 1, a=HPC)[
                :, mt * HPC : (mt + 1) * HPC, 0:DH
            ],
            ps[:].rearrange("p (a c) -> p a c", c=DH),
        )

    # q/k APs per head: (tile, row0)
    QAP = {0: (qA, 0), 1: (qA, DH), 2: (tB, 0)}
    KAP = {0: (kA, 0), 1: (kA, DH), 2: (kB, 0)}

    def logits(h, w, mt):
        qt, qr = QAP[h]
        kt_, kr = KAP[h]
        ps = pa.tile([P, W], f32, tag="pa", name="ps_l")
        for s in range(W // 512):
            nc.tensor.matmul(
                ps[:, s * 512 : (s + 1) * 512],
                kt_[kr : kr + DH, mt * P : (mt + 1) * P],
                qt[qr : qr + DH, w * W + s * 512 : w * W + (s + 1) * 512],
                start=True,
                stop=True,
            )
        return ps

    def expf(ps, mt):
        et = exps.tile([P, W], bf16, tag="exp", name="et")
        nc.scalar.activation(
            et[:], ps[:], mybir.ActivationFunctionType.Exp,
            bias=mb_sb[:, mt : mt + 1], scale=SCALE,
        )
        return et

    # exp on DVE (2 passes) for tiles the Activation engine can't absorb.
    # pass 1 (psum -> int32) runs inline to free the psum slot; the fused
    # correction pass is deferred (via pending_dve) out of the norm window.
    pending_dve = []

    def expf_dve(ps, mt):
        it = big.tile([P, W], i32, tag="eit", name="eit", bufs=4)
        nc.vector.tensor_scalar(
            it[:], ps[:], SCALE * EXP_SCALE, mb2_sb[:, mt : mt + 1],
            mybir.AluOpType.mult, mybir.AluOpType.add,
        )
        et = exps.tile([P, W], bf16, tag="exp", name="et")

        def fix(it=it, et=et):
            nc.vector._custom_dve(
                _get_exp_fix(), out=et[:], in0=it[:].bitcast(f32),
                s0=MANT_MASK, s1=EXP_C,
            )
        pending_dve.append(fix)
        return et

    def flush_dve():
        while pending_dve:
            pending_dve.pop(0)()

    # one PV chain: 128 queries (tile j of half w) x [v_h | ones]
    def et_ap(e, j):
        if isinstance(e, tuple):  # split 512-wide pair from the warm start
            e = e[j // (JT // 2)]
            j = j % (JT // 2)
        return e[:, j * P : (j + 1) * P]

    def chain(ets, h, j):
        cps = ch.tile([P, DH + 1], f32, tag="ch", name="cps")
        for mt in range(NMT):
            nc.tensor.matmul(
                cps[:, :],
                et_ap(ets[mt], j),
                v_sb[:, mt * VB + h * (DH + 1) : mt * VB + (h + 1) * (DH + 1)],
                start=(mt == 0),
                stop=(mt == NMT - 1),
            )
        return cps

    # normalize chain j of head h into the norm tile for (pair, w)
    def norm(cps, dest_ap):
        rc = rcs.tile([P, 1], f32, tag="rc", name="rc")
        nc.vector.reciprocal(rc[:], cps[:, DH : DH + 1])
        nc.vector.tensor_scalar_mul(dest_ap, cps[:, 0:DH], rc[:])

    def proj_mm(nt):
        ps_y = pa.tile([P, W], f32, tag="pa", name="ps_y")
        for o0, ow in ((0, 512), (512, 256)):
            nc.tensor.matmul(
                ps_y[:, o0 : o0 + ow],
                atA[:, nt * P : (nt + 1) * P],
                wpA[:, o0 : o0 + ow],
                start=True,
                stop=False,
            )
            nc.tensor.matmul(
                ps_y[:, o0 : o0 + ow],
                atB[:, nt * P : (nt + 1) * P],
                wpB[:, o0 : o0 + ow],
                start=False,
                stop=True,
            )
        return ps_y

    def proj(nt, copy_eng=None):
        ps_y = proj_mm(nt)
        yt = ys.tile([P, C], bf16, tag="y", name="yt")
        if copy_eng == "act":
            nc.scalar.copy(yt[:], ps_y[:, :C])
        else:
            nc.vector.tensor_copy(yt[:], ps_y[:, :C])
        nc.sync.dma_start(y[nt * P : (nt + 1) * P, :], yt[:])

    # tail variant: projections land in a strip; one batched DMA per 4 tiles
    def proj_strip(nt, strip, idx, copy_eng):
        ps_y = proj_mm(nt)
        dest = strip[:, idx * C : (idx + 1) * C]
        if copy_eng == "act":
            nc.scalar.copy(dest, ps_y[:, :C])
        else:
            nc.vector.tensor_copy(dest, ps_y[:, :C])

    # ---------------- schedule ----------------
    # units: (head, half); unit u's loop hosts unit u-1's chains as filler
    UNITS = [(0, 0), (1, 0), (2, 0), (0, 1), (1, 1), (2, 1)]
    CH0 = 4  # chains of the previous unit run at mt slots CH0..CH0+7

    warmup(8)
    # prologue: just enough q/k for the first 512-wide logits half
    qk_pass(0, P, 0, 256, qA)
    qk_pass(P, P, 0, 256, kA)

    # unit-0 warm start: two 512-wide logits/exp halves for mt 0, emitted
    # interleaved with the remaining q passes so exp starts as soon as the
    # first x chunks land
    def unit0_mt0():
        psA = pa.tile([P, 512], f32, tag="pa", name="ps_l5")
        nc.tensor.matmul(psA[:, 0:256], kA[0:DH, 0:P], qA[0:DH, 0:256],
                         start=True, stop=True)
        qk_pass(0, P, 256, 512, qA)
        nc.tensor.matmul(psA[:, 256:512], kA[0:DH, 0:P], qA[0:DH, 256:512],
                         start=True, stop=True)
        etA = exps.tile([P, 512], bf16, tag="exp", name="et")
        nc.scalar.activation(etA[:], psA[:], mybir.ActivationFunctionType.Exp,
                             bias=mb_sb[:, 0:1], scale=SCALE)
        qk_pass(P, P, 256, 512, kA)
        psB = pa.tile([P, 512], f32, tag="pa", name="ps_l5")
        qk_pass(0, P, 512, 768, qA)
        nc.tensor.matmul(psB[:, 0:256], kA[0:DH, 0:P], qA[0:DH, 512:768],
                         start=True, stop=True)
        qk_pass(0, P, 768, 1024, qA)
        nc.tensor.matmul(psB[:, 256:512], kA[0:DH, 0:P], qA[0:DH, 768:1024],
                         start=True, stop=True)
        etB = exps.tile([P, 512], bf16, tag="exp", name="et")
        nc.scalar.activation(etB[:], psB[:], mybir.ActivationFunctionType.Exp,
                             bias=mb_sb[:, 0:1], scale=SCALE)
        return (etA, etB)

    def mk_pass(col0, rows, c, dest):
        return lambda: qk_pass(col0, rows, c * 512, (c + 1) * 512, dest)

    def kb_dma(c):
        # move k2 of chunk c from tB rows 64:128 to kB rows 0:64
        return lambda: nc.sync.dma_start(
            kB[:, c * 512 : (c + 1) * 512], tB[DH:P, c * 512 : (c + 1) * 512]
        )

    def mk_half(col0, rows, c, half, dest):
        n0 = c * 512 + half * 256
        return lambda: qk_pass(col0, rows, n0, n0 + 256, dest)

    fillers = {u: [] for u in range(len(UNITS))}
    # units 0-1: remaining k chunks (ahead of their key tiles), the combined
    # q2|k2 chunks + k2 base moves, and all v tiles (before unit 0's chains)
    fillers[0] = [
        (1, mk_half(P, P, 1, 0, kA)),    # k tiles 4..7 before mt 4
        (2, mk_half(P, P, 1, 1, kA)),
        (3, lambda: v_tile(0)),
        (4, mk_half(2 * P, P, 0, 0, tB)),
        (5, mk_half(2 * P, P, 0, 1, tB), kb_dma(0)),
        (6, mk_half(P, P, 2, 0, kA)),    # k tiles 8..11 before mt 8
        (7, mk_half(P, P, 2, 1, kA)),
        (8, mk_half(2 * P, P, 1, 0, tB)),
        (9, mk_half(2 * P, P, 1, 1, tB), kb_dma(1)),
        (10, mk_half(P, P, 3, 0, kA)),   # k tiles 12..15 before mt 12
        (11, mk_half(P, P, 3, 1, kA)),
        (12, lambda: v_tile(1)),
        (13, lambda: v_tile(2)),
        (14, lambda: v_tile(3)),
        (15, lambda: v_tile(4)),
    ]
    fillers[1] = [
        (0, mk_half(2 * P, P, 2, 0, tB)),
        (1, mk_half(2 * P, P, 2, 1, tB), kb_dma(2)),
        (2, lambda: v_tile(5), lambda: v_tile(6)),
        (3, lambda: v_tile(7), lambda: v_tile(8), lambda: v_tile(9)),
        (4, lambda: v_tile(10), lambda: v_tile(11), lambda: v_tile(12)),
        (5, lambda: v_tile(13), lambda: v_tile(14), lambda: v_tile(15)),
        (13, mk_pass(2 * P, P, 3, tB)),
        (14, kb_dma(3)),
    ]
    fillers[2] = [
        (0, mk_pass(0, P, 2, qA)),       # q half 1 for unit 3
        (1, mk_pass(0, P, 3, qA)),
    ]
    fillers[3] = [
        (13, lambda: proj(0)), (14, lambda: proj(1)), (15, lambda: proj(2)),
    ]
    fillers[4] = [
        (0, lambda: proj(3, "act")), (1, lambda: proj(4, "act")),
        (2, lambda: proj(5, "act")), (3, lambda: proj(6)),
        (12, lambda: proj(7)),
    ]

    norm_tiles = {}

    def norm_dest(h, w, j):
        # pair tile for (h0,h1); own tile for h2 (pad cols stay zero)
        key = ("A" if h < 2 else "B", w)
        if key not in norm_tiles:
            t = norms.tile([P, W], bf16, tag="nt" + key[0], name="ntile")
            if key[0] == "B":
                nc.gpsimd.memset(t[:], 0.0)
            norm_tiles[key] = t
        t = norm_tiles[key]
        off = j * P + (DH if h == 1 else 0)
        return t[:, off : off + DH]

    def transpose_cols(key, w, dest, rows, c0, c1, eng=None):
        t = norm_tiles[(key, w)]
        (eng or nc.sync).dma_start_transpose(
            dest[0:rows, w * W + c0 : w * W + c1].rearrange(
                "p (b q) -> p b q", q=P
            ),
            t[:, c0:c1],
        )

    def transpose_pair(key, w, dest, rows):
        transpose_cols(key, w, dest, rows, 0, W)
        del norm_tiles[(key, w)]

    # exp tiles routed to DVE in units where the PE has slack and the
    # Activation engine is the limiter (slots outside the chain-hosting
    # window so the DVE pass runs promptly and frees the psum slot)
    DVE_MTS = {2: (1, 3), 3: (1, 3, 13), 4: (1, 3, 13), 5: (1, 3, 13)}

    prev = None  # (head, half, ets) of previous unit
    for u, (h, w) in enumerate(UNITS):
        ets = []
        fill = list(fillers[u])
        dve_mts = DVE_MTS.get(u, ())
        ch0 = 6 if u == 1 else CH0
        for mt in range(NMT):
            if u == 0 and mt == 0:
                ets.append(unit0_mt0())
                continue
            ps = logits(h, w, mt)
            ets.append(expf_dve(ps, mt) if mt in dve_mts else expf(ps, mt))
            while fill and fill[0][0] <= mt:
                for fn in fill.pop(0)[1:]:
                    fn()
            if prev is not None and ch0 <= mt < ch0 + JT:
                ph, pw, pets = prev
                j = mt - ch0
                cps = chain(pets, ph, j)
                norm(cps, norm_dest(ph, pw, j))
                if ph == 1 and j == JT - 1:
                    transpose_pair("A", pw, atA, P)
                if ph == 2 and j == JT - 1:
                    transpose_pair("B", pw, atB, DH)
        flush_dve()
        prev = (h, w, ets)

    # tail: chains of the last unit (h2, w1) with mini transposes issued as
    # each pair completes, then the remaining projections back to back (all
    # transposes but the last have completed by the time proj needs them)
    ph, pw, pets = prev
    for j in range(JT):
        cps = chain(pets, ph, j)
        norm(cps, norm_dest(ph, pw, j))
        if j % 2 == 1:
            transpose_cols("B", pw, atB, DH, (j - 1) * P, (j + 1) * P,
                           eng=nc.scalar)
    del norm_tiles[("B", pw)]
    for half in range(2):
        strip = ys.tile([P, 4 * C], bf16, tag="ystrip", name="ystrip", bufs=2)
        for i in range(4):
            nt = JT + half * 4 + i
            proj_strip(nt, strip, i, "act" if i % 2 == 0 else "vec")
        nc.sync.dma_start(
            y[(JT + half * 4) * P : (JT + half * 4 + 4) * P, :].rearrange(
                "(t p) c -> p t c", p=P
            ),
            strip[:].rearrange("p (t c) -> p t c", c=C),
        )


def _get_nc(reps=1):
    if reps not in _nc_cache:
        _nc_cache[reps] = _build(reps)
    return _nc_cache[reps]


def prep_in_maps(x, att_mask, qkv_w, proj_w):
    """Per-core input prep (host): slice heads, transpose, cast to bf16."""
    in_maps = []
    for b in range(B):
        xT = np.ascontiguousarray(x[b].T).astype(ml_dtypes.bfloat16)
        mb = np.where(att_mask[b] == 0, -1e30, 0.0).astype(np.float32)
        mbias = np.ascontiguousarray(mb.reshape(NMT, P).T)
        mbias2 = (mbias.astype(np.float64) * EXP_SCALE + EXP_BIAS).astype(np.float32)
        for g in range(G):
            r0 = g * HPC * DH
            r1 = (g + 1) * HPC * DH
            wq = qkv_w[r0:r1]
            wk = qkv_w[C + r0 : C + r1]
            wv = qkv_w[2 * C + r0 : 2 * C + r1]
            wqkT = np.ascontiguousarray(
                np.concatenate(
                    [wq[0 : 2 * DH], wk[0 : 2 * DH], wq[2 * DH :], wk[2 * DH :]], 0
                ).T
            ).astype(ml_dtypes.bfloat16)
            wvT = np.ascontiguousarray(wv.T).astype(ml_dtypes.bfloat16)
            wpT = np.ascontiguousarray(proj_w[:, r0:r1].T)
            in_maps.append(
                {
                    "xT": xT,
                    "wqkT": wqkT,
                    "wvT": wvT,
                    "wpTA": wpT[0 : 2 * DH].astype(ml_dtypes.bfloat16),
                    "wpTB": wpT[2 * DH :].astype(ml_dtypes.bfloat16),
                    "mbias": mbias,
                    "mbias2": mbias2,
                }
            )
    return in_maps


def kernel(x, att_mask, qkv_w, proj_w, proj_b):
    global LAST_EXEC_NS, LAST_RESULTS
    x = np.asarray(x, dtype=np.float32)
    att_mask = np.asarray(att_mask)
    qkv_w = np.asarray(qkv_w, dtype=np.float32)
    proj_w = np.asarray(proj_w, dtype=np.float32)
    proj_b = np.asarray(proj_b, dtype=np.float32)

    nc = _get_nc()
    in_maps = prep_in_maps(x, att_mask, qkv_w, proj_w)

    res = run_bass_kernel_spmd(
        nc, in_maps, core_ids=list(range(B * G)), trace=TRACE
    )
    LAST_EXEC_NS = res.exec_time_ns
    LAST_RESULTS = res

    out = np.zeros((B, N, C), np.float32)
    for b in range(B):
        acc = np.asarray(res.results[b * G]["y"]).astype(np.float32)
        for g in range(1, G):
            acc += np.asarray(res.results[b * G + g]["y"]).astype(np.float32)
        out[b] = acc + proj_b[None, :]
    return out
